# revision 33
# baseline (speedup 1.0000x reference)
"""DecoderLSTM (Bahdanau attention + LSTM + vocab fc) on 8 Trainium2 cores.

Sharding: data-parallel over batch (64 -> 8 rows/core); the sequential scan
stays local per core; zero collectives. Host shards/casts/transposes inputs
and reassembles (f16 logits -> f32 + fc_b on host).

Feature-major recurrence: state h2T/c2T kept transposed [a-tiles, 8], so all
LSTM-side matmuls have n=8 (PE cost ~ output free size), and the pointwise
ops run on [128, 4, 8] tiles.

v1 changes over the 350us baseline:
- scores via fp8 DoubleRow matmuls: X = tanh(...) written as e4m3, diagonal
  energy-weight trick packed per at-pair [128,2,8]; exp gets scale=1/16 to
  undo the x16 pre-scale that keeps ew out of fp8 denormals.
- broadcast adds (ept + decT) as single DVE tensor_tensor ops reading the
  dec PSUM directly (at0-2) + one Pool op (at3); no per-b tensor_scalar.
- gates x-part precomputed once into gx (PE identity-matmul per step folds
  it into the PSUM accumulation) instead of 5 matmuls per g-tile per step.
- fc m0 (rows 0..127) in 2-chunk drain groups: one [128,1000] drain copy +
  one DMA each; fetch in 4-chunk [128,4,2000] group DMAs.
- fc m1 (rows 128..151) computed TRANSPOSED at the tail: out [125-vtile, 24]
  per group, 64 accumulating matmuls per bank, one [125,384] drain + DMA to
  d_outT; host re-transposes. Kills the 33us serial m-major tail.
- no gpsimd-issued DMAs (Pool engine was burning 1us per issue).
"""

import numpy as np
import ml_dtypes

import concourse.bass as bass
import concourse.bacc as bacc
import concourse.tile as tile
from concourse import mybir
from concourse.bass_utils import run_bass_kernel_spmd

F16 = mybir.dt.float16
F32 = mybir.dt.float32
F8 = mybir.dt.float8e4
F8E3 = mybir.dt.float8e3
E4M3 = ml_dtypes.float8_e4m3
E3M4 = ml_dtypes.float8_e3m4
DR = mybir.MatmulPerfMode.DoubleRow

B, N, H, E, A, V, L = 64, 196, 512, 512, 512, 20000, 20
T = L - 1            # 19 decode steps
NC = 8               # cores
BS = B // NC         # 8 batch rows per core
BN = BS * N          # 1568
BT = T * BS          # 152 rows, t-major (row = t*8 + b)
VC = 500             # fc vocab chunk width
NCH = V // VC        # 40
NFG = 10             # fetch groups (4 chunks each)
VT = 125             # m1-transposed vocab tile width (16 per fetch group)
EWS = 16.0           # energy_W pre-scale (undone by exp scale)
FCS = 32.0           # fc_W pre-scale into e3m4 normal range (undone on host)

# gate reorder [i,f,g,o] -> [i,f,o,g] so tanh(0.5*x) covers cols 0:1536
PERM = np.concatenate([np.arange(0, H), np.arange(H, 2 * H),
                       np.arange(3 * H, 4 * H), np.arange(2 * H, 3 * H)])

TANH = mybir.ActivationFunctionType.Tanh
EXP = mybir.ActivationFunctionType.Exp
ADD = mybir.AluOpType.add
MULT = mybir.AluOpType.mult


def prep_core(core, inputs):
    """Per-core numpy input dict (shard + transpose + cast only)."""
    f32 = np.float32
    bsl = slice(core * BS, (core + 1) * BS)
    enc = np.asarray(inputs["encoder_outputs"][bsl], f32)      # [8,196,512]

    enc_t = np.ascontiguousarray(enc.reshape(BN, H).T).astype(np.float16)
    enc_r = np.zeros((2 * BS, 128, H), np.float16)
    for b in range(BS):
        enc_r[2 * b, :128] = enc[b, :128]
        enc_r[2 * b + 1, :N - 128] = enc[b, 128:]

    caps = np.asarray(inputs["captions"][bsl])[:, :T]          # [8,19]
    es = np.asarray(inputs["emb"], f32)[caps]                  # [8,19,512]
    emb_flat = es.transpose(1, 0, 2).reshape(BT, E)            # t-major rows
    emb_cat = np.concatenate(
        [emb_flat.T, np.ones((1, BT), f32)], 0).astype(np.float16)

    wih = np.asarray(inputs["W_ih"], f32)[PERM]                # [2048,1024]
    whh = np.asarray(inputs["W_hh"], f32)[PERM]
    bias = (np.asarray(inputs["b_ih"], f32) +
            np.asarray(inputs["b_hh"], f32))[PERM]
    wihxb_t = np.concatenate(
        [wih[:, :E].T, bias[None, :]], 0)                      # [513,2048]
    wc_t = np.concatenate([wih[:, E:].T, 0.5 * whh.T], 0)      # [1024,2048]
    # pre-double the g-gate columns so the pointwise uses one tanh(0.5 x)
    wihxb_t[:, 3 * H:] *= 2.0
    wc_t[:, 3 * H:] *= 2.0
    wihxb_t = wihxb_t.astype(np.float16)
    wc_t = wc_t.astype(np.float16)

    dec_wt = (0.5 * np.asarray(inputs["dec_W"], f32).T).astype(np.float16)
    enc_wt = np.ascontiguousarray(
        np.asarray(inputs["enc_W"], f32).T).astype(np.float16)  # [H,A]
    ebdb = np.ascontiguousarray(
        (np.asarray(inputs["enc_b"], f32) +
         np.asarray(inputs["dec_b"], f32)).reshape(4, 128).T)   # [128,4]
    # diagonal energy-weight trick, fp8, at-pair packed, per b-group:
    # [128, 2ap, 2i, 2grp, 16]; nonzero at col bl4*4+bl4 (4x4 diagonal)
    ew8 = (EWS * np.asarray(inputs["energy_W"], f32)[0]).astype(E4M3)
    ewm8 = np.zeros((128, 2, 2, 2, 16), E4M3)
    for ap in range(2):
        for i in range(2):
            seg = ew8[ap * 256 + i * 128: ap * 256 + (i + 1) * 128]
            for g in range(2):
                ewm8[:, ap, i, g, 0:16:5] = np.repeat(seg[:, None], 4, 1)
    fcw_t = np.ascontiguousarray(
        0.5 * FCS * np.asarray(inputs["fc_W"], f32).T).astype(E3M4)
    id8 = np.eye(8, dtype=np.float16)
    id128 = np.eye(128, dtype=np.float16)

    return {"enc_t": enc_t, "enc_r": enc_r, "emb_cat": emb_cat,
            "wihxb_t": wihxb_t, "wc_t": wc_t, "dec_wt": dec_wt,
            "enc_wt": enc_wt, "ebdb": ebdb, "ewm8": ewm8,
            "fcw_t": fcw_t, "id8": id8, "id128": id128}


def _bcast(ap, n):
    """Append an innermost step-0 (broadcast) dim of size n to an AP."""
    return bass.AP(tensor=ap.tensor, offset=ap.offset,
                   ap=list(ap.ap) + [[0, n]])


def build_program():
    nc = bacc.Bacc("TRN2", target_bir_lowering=False, debug=False,
                   num_devices=NC)
    d_enc_t = nc.dram_tensor("enc_t", [H, BN], F16, kind="ExternalInput")
    d_enc_r = nc.dram_tensor("enc_r", [2 * BS, 128, H], F16,
                             kind="ExternalInput")
    d_emb = nc.dram_tensor("emb_cat", [E + 1, BT], F16, kind="ExternalInput")
    d_wx = nc.dram_tensor("wihxb_t", [E + 1, 4 * H], F16,
                          kind="ExternalInput")
    d_wc = nc.dram_tensor("wc_t", [2 * H, 4 * H], F16, kind="ExternalInput")
    d_dwt = nc.dram_tensor("dec_wt", [H, A], F16, kind="ExternalInput")
    d_ewt = nc.dram_tensor("enc_wt", [H, A], F16, kind="ExternalInput")
    d_ebdb = nc.dram_tensor("ebdb", [128, 4], F32, kind="ExternalInput")
    d_ewm8 = nc.dram_tensor("ewm8", [128, 2, 2, 2, 16], F8,
                            kind="ExternalInput")
    d_fcw = nc.dram_tensor("fcw_t", [H, V], F8E3, kind="ExternalInput")
    d_id8 = nc.dram_tensor("id8", [8, 8], F16, kind="ExternalInput")
    d_id128 = nc.dram_tensor("id128", [128, 128], F16, kind="ExternalInput")
    d_out = nc.dram_tensor("logits", [128, V], F16, kind="ExternalOutput")
    d_outT = nc.dram_tensor("logitsT", [V, 24], F16, kind="ExternalOutput")

    with tile.TileContext(nc) as tc:
        _build_body(nc, tc, d_enc_t, d_enc_r, d_emb, d_wx, d_wc, d_dwt,
                    d_ewt, d_ebdb, d_ewm8, d_fcw, d_id8, d_id128,
                    d_out, d_outT)
    nc.compile()
    return nc


def _build_body(nc, tc, d_enc_t, d_enc_r, d_emb, d_wx, d_wc, d_dwt, d_ewt,
                d_ebdb, d_ewm8, d_fcw, d_id8, d_id128, d_out, d_outT):
    with tc.tile_pool(name="res", bufs=1) as res:
        # -------- residents --------
        ept = res.tile([128, 4, BN], F16)        # enc_projT (+enc_b+dec_b)
        enr = res.tile([128, 2 * BS, H], F16)    # enc rows [n-tiles, h]
        wc = res.tile([128, 8, 4 * H], F16)      # [ctx;h] gate weights^T
        dwt = res.tile([128, 4, A], F16)         # 0.5 dec_W^T
        gx = res.tile([128, 16, BT], F16)        # precomputed x-part gates
        hallt = res.tile([128, 4, BT], F16)      # h2^T, all steps
        xt8 = res.tile([128, 4, BN], F8)         # tanh(X) in fp8 (scores rhs)
        ewm8 = res.tile([128, 2, 2, 2, 16], F8)  # diag ew, grouped
        id8 = res.tile([8, 8], F16)
        id128 = res.tile([128, 128], F16)
        c2 = res.tile([128, 4, 8], F32)          # 2c, transposed
        ebdb = res.tile([128, 4], F32)
        fcpre = res.tile([128, 4, V], F8E3)
        emb = res.tile([128, 5, BT], F16)        # emb_cat k-tiles (+ones row)
        wx = res.tile([128, 5, 4 * H], F16)      # W_ih_x^T (+bias row)

        # tiny first, then DMAs ordered by first use
        nc.sync.dma_start(out=id8[:, :], in_=d_id8[:, :])
        nc.sync.dma_start(out=ebdb[:, :], in_=d_ebdb[:, :])
        nc.sync.dma_start(out=ewm8[:, :, :, :, :],
                          in_=d_ewm8[:, :, :, :, :])
        nc.sync.dma_start(out=id128[:, :], in_=d_id128[:, :])
        nc.vector.memset(c2[:, :, :], 0.0)
        z1 = res.tile([1, 512], F16)
        nc.vector.memset(z1[:, :], 0.0)
        zf = res.tile([1, 512], F32)
        nc.vector.memset(zf[:, :], 0.0)
        # touch Tanh+Exp early so the one-time activation table load (1.3us)
        # happens during the setup DMAs, not on step 0's critical path
        warm = res.tile([1, 8], F16)
        nc.scalar.activation(out=warm[0:1, 0:8], in_=z1[0:1, 0:8], func=TANH)
        nc.scalar.activation(out=warm[0:1, 0:8], in_=z1[0:1, 0:8], func=EXP)

        def flat(tile_ap, ncols):
            return tile_ap.rearrange("p a b -> p (a b)")

        def bank_open(ap, ncols, dep=None, npart=128):
            # full-tile zero matmul: zeroes the 2KB bank and starts its single
            # accumulation group. lhsT is all-zero so the rhs (an optional
            # SBUF tile produced by the op this must execute after) only
            # carries a read dependency.
            rhs = z1[0:1, 0:ncols] if dep is None else dep
            lhs = zf if rhs.dtype == F32 else z1
            nc.tensor.matmul(ap, lhs[0:1, 0:npart], rhs,
                             start=True, stop=False)

        def bank_close(ap, ncols, dep=None, npart=128):
            rhs = z1[0:1, 0:ncols] if dep is None else dep
            lhs = zf if rhs.dtype == F32 else z1
            nc.tensor.matmul(ap, lhs[0:1, 0:npart], rhs,
                             start=False, stop=True)

        # -------- setup: enc_projT (+enc_b+dec_b), gates-x precompute ------
        with tc.tile_pool(name="se", bufs=1) as se, \
             tc.tile_pool(name="sep", bufs=2, space="PSUM") as sep:
            et = se.tile([128, 4, BN], F16)
            ewt = se.tile([128, 4, A], F16)
            nc.sync.dma_start(
                out=ewt[:, :, :],
                in_=d_ewt[:, :].rearrange("(k p) a -> p k a", p=128))
            for ch in range(4):                      # chunked: MMs start early
                nc.sync.dma_start(
                    out=et[:, :, ch * 392:(ch + 1) * 392],
                    in_=d_enc_t[:, ch * 392:(ch + 1) * 392].rearrange(
                        "(k p) n -> p k n", p=128))
            nc.sync.dma_start(
                out=dwt[:, :, :],
                in_=d_dwt[:, :].rearrange("(k p) a -> p k a", p=128))
            nc.sync.dma_start(
                out=emb[:, 0:4, :],
                in_=d_emb[0:512, :].rearrange("(k p) t -> p k t", p=128))
            nc.sync.dma_start(out=emb[0:1, 4, :], in_=d_emb[512:513, :])
            nc.sync.dma_start(
                out=wx[:, 0:4, :],
                in_=d_wx[0:512, :].rearrange("(k p) g -> p k g", p=128))
            nc.sync.dma_start(out=wx[0:1, 4, :], in_=d_wx[512:513, :])
            nc.sync.dma_start(out=enr[:, :, :],
                              in_=d_enc_r[:, :, :].rearrange("j p h -> p j h"))
            # wc: ctx half (k 0:4) first (needed at t0), then h half
            nc.sync.dma_start(
                out=wc[:, 0:4, :],
                in_=d_wc[0:512, :].rearrange("(k p) g -> p k g", p=128))
            nc.sync.dma_start(
                out=wc[:, 4:8, :],
                in_=d_wc[512:1024, :].rearrange("(k p) g -> p k g", p=128))
            for q in range(4):                    # whole fc_W, fp8 resident
                nc.sync.dma_start(
                    out=fcpre[:, :, q * 5000:(q + 1) * 5000],
                    in_=d_fcw[:, q * 5000:(q + 1) * 5000].rearrange(
                        "(k p) v -> p k v", p=128))
            for at in range(4):                      # a-tile = out m-tile
                for ch in range(4):                  # 1568 = 4*392
                    pt = sep.tile([128, 392], F32, tag="sep")
                    for k in range(4):
                        nc.tensor.matmul(
                            pt[:, :],
                            ewt[:, k, at * 128:(at + 1) * 128],
                            et[:, k, ch * 392:(ch + 1) * 392],
                            start=(k == 0), stop=(k == 3))
                    # alternate the bias-add drain across DVE and ACT so
                    # neither engine serializes the whole ept production
                    osl = ept[:, at, ch * 392:(ch + 1) * 392]
                    if ch % 2 == 0:
                        nc.vector.tensor_scalar_add(
                            out=osl, in0=pt[:, :],
                            scalar1=ebdb[:, at:at + 1])
                    else:
                        nc.scalar.activation(
                            out=osl, in_=pt[:, :],
                            func=mybir.ActivationFunctionType.Identity,
                            bias=ebdb[:, at:at + 1])
            # gates-x: gx[:, g, :] = (wihxb^T emb_cat); only the first 5
            # steps' columns here -- the rest weaves into steps 0-1
            for g in range(16):
                gsl = slice(g * 128, (g + 1) * 128)
                ptg = sep.tile([128, 40], F32, tag="sep")
                for k in range(5):
                    kr = 128 if k < 4 else 1
                    nc.tensor.matmul(ptg[:, :], wx[0:kr, k, gsl],
                                     emb[0:kr, k, 0:40],
                                     start=(k == 0), stop=(k == 4))
                nc.vector.tensor_copy(out=gx[:, g, 0:40], in_=ptg[:, :])

        # -------- recurrence, 2-way b-split software pipeline --------
        # Groups G0 = b 0..3, G1 = b 4..7 run their serial attention/LSTM
        # chains staggered so ACT's tanh burst for one group overlaps the
        # other group's softmax/ctx/gates glue. fc m0 chunks weave into PE
        # idle at t>=16 (1-bank jobs, 2 PSUM bufs ping-pong).
        m0_jobs = list(range(NCH))

        with tc.tile_pool(name="pdx0", bufs=1, space="PSUM") as pdx0, \
             tc.tile_pool(name="pdx1", bufs=1, space="PSUM") as pdx1, \
             tc.tile_pool(name="pg0", bufs=1, space="PSUM") as pg0, \
             tc.tile_pool(name="pg1", bufs=1, space="PSUM") as pg1, \
             tc.tile_pool(name="ps0", bufs=1, space="PSUM") as ps0, \
             tc.tile_pool(name="ps1", bufs=1, space="PSUM") as ps1, \
             tc.tile_pool(name="pf", bufs=2, space="PSUM") as pfp, \
             tc.tile_pool(name="stp", bufs=2) as stp, \
             tc.tile_pool(name="fo", bufs=3) as fop, \
             tc.tile_pool(name="xp", bufs=1) as xp:

            pdx = [pdx0, pdx1]
            pgt = [pg0, pg1]
            psp = [ps0, ps1]
            fc_pending = []

            def m0_mm(ch):
                pf = pfp.tile([128, VC], F32, tag="pf", name=f"pf{ch}")
                for k in range(4):
                    nc.tensor.matmul(
                        pf[:, :], hallt[:, k, 0:128],
                        fcpre[:, k, ch * VC:(ch + 1) * VC],
                        start=(k == 0), stop=(k == 3))
                fc_pending.append((pf, ch))

            def m0_drain(eng, limit=None):
                nd = len(fc_pending) if limit is None else \
                    min(limit, len(fc_pending))
                for _ in range(nd):
                    pf, ch = fc_pending.pop(0)
                    fo = fop.tile([128, VC], F16, tag="fo", name=f"fo{ch}")
                    if eng is nc.scalar:
                        eng.copy(out=fo[:, :], in_=pf[:, :])
                    else:
                        eng.tensor_copy(out=fo[:, :], in_=pf[:, :])
                    nc.sync.dma_start(
                        out=d_out[:, ch * VC:(ch + 1) * VC], in_=fo[:, :])

            def gx_pass(g0, g1, c0, c1):
                for g in range(g0, g1):
                    gsl = slice(g * 128, (g + 1) * 128)
                    ptg = pfp.tile([128, c1 - c0], F32, tag="pf",
                                   name=f"gxb{g}_{c0}")
                    for k in range(5):
                        kr = 128 if k < 4 else 1
                        nc.tensor.matmul(ptg[:, :], wx[0:kr, k, gsl],
                                         emb[0:kr, k, c0:c1],
                                         start=(k == 0), stop=(k == 4))
                    nc.vector.tensor_copy(out=gx[:, g, c0:c1], in_=ptg[:, :])

            GW = 4 * N  # 784: one b-group's column width in (b n) layouts

            def g_dec(i, t):
                """dec matmuls for group i -> dps_i [128, 4at, 4b]."""
                dps = pdx[i].tile([128, 4, 4], F32, tag="dx",
                                  name=f"dec{t}_{i}")
                hsl = slice(t * 8 + 4 * i - 8, t * 8 + 4 * i - 4)
                bank_open(flat(dps, 16), 16,
                          dep=hallt[0:1, :, hsl])
                for at in range(4):
                    for k in range(4):
                        nc.tensor.matmul(
                            dps[:, at, :],
                            dwt[:, k, at * 128:(at + 1) * 128],
                            hallt[:, k, hsl], start=False, stop=False)
                bank_close(flat(dps, 16), 16)
                return dps

            def g_gates_h(i, t):
                gps = pgt[i].tile([128, 16, 4], F32, tag="g",
                                  name=f"g{t}_{i}")
                hsl = slice(t * 8 + 4 * i - 8, t * 8 + 4 * i - 4)
                xsl = slice(t * 8 + 4 * i, t * 8 + 4 * i + 4)
                bank_open(flat(gps, 64), 64)
                for g in range(16):
                    gsl = slice(g * 128, (g + 1) * 128)
                    if t > 0:
                        for k in range(4):
                            nc.tensor.matmul(gps[:, g, :], wc[:, 4 + k, gsl],
                                             hallt[:, k, hsl],
                                             start=False, stop=False)
                    nc.tensor.matmul(gps[:, g, :], id128[:, :],
                                     gx[:, g, xsl], start=False, stop=False)
                return gps

            def g_adds(i, t, dps):
                """ept + dec broadcast: DVE at0/at2 (PSUM direct), Pool
                at1/at3 (via dect copies)."""
                dect = stp.tile([128, 2, 4], F32, tag=f"dect{i}",
                                name=f"dect{t}_{i}")
                nc.vector.tensor_copy(out=dect[:, 0, :], in_=dps[:, 1, :])
                nc.vector.tensor_copy(out=dect[:, 1, :], in_=dps[:, 3, :])
                xas = []
                for ap in range(2):
                    xa = xp.tile([128, 2, GW], F16, tag=f"x{i}_{ap}",
                                 name=f"xa{t}_{i}{ap}")
                    xas.append(xa)
                for at in (1, 0, 3, 2):
                    ap, j = at // 2, at % 2
                    src = ept[:, at, i * GW:(i + 1) * GW].rearrange(
                        "p (b n) -> p b n", n=N)
                    dst = xas[ap][:, j, :].rearrange("p (b n) -> p b n", n=N)
                    if j == 0:
                        nc.vector.tensor_add(
                            out=dst, in0=src, in1=_bcast(dps[:, at, :], N))
                    else:
                        nc.gpsimd.tensor_add(
                            out=dst, in0=src,
                            in1=_bcast(dect[:, at // 2, :], N))
                return xas

            def g_tanh_scores(i, t, xas):
                psc = psp[i].tile([4, N], F32, tag="sc", name=f"sc{t}_{i}")
                for ap in range(2):
                    if t == 0:
                        nc.scalar.activation(
                            out=xt8[:, 2 * ap:2 * ap + 2,
                                    i * GW:(i + 1) * GW],
                            in_=ept[:, 2 * ap:2 * ap + 2,
                                    i * GW:(i + 1) * GW], func=TANH)
                    else:
                        nc.scalar.activation(
                            out=xt8[:, 2 * ap:2 * ap + 2,
                                    i * GW:(i + 1) * GW],
                            in_=xas[ap][:, :, :], func=TANH)
                for ap in range(2):
                    for bl in range(4):
                        nc.tensor.matmul(
                            psc[:, :],
                            ewm8[:, ap, :, i, bl * 4:(bl + 1) * 4],
                            xt8[:, 2 * ap:2 * ap + 2,
                                i * GW + bl * N:i * GW + (bl + 1) * N],
                            start=(ap == 0 and bl == 0),
                            stop=(ap == 1 and bl == 3),
                            perf_mode=DR)
                return psc

            def g_softmax(i, t, psc):
                atw = stp.tile([4, N], F16, tag=f"atw{i}")
                zs = stp.tile([4, 1], F32, tag=f"zs{i}")
                nc.scalar.activation(out=atw[:, :], in_=psc[:, :],
                                     func=EXP, scale=1.0 / EWS,
                                     accum_out=zs[:, 0:1])
                rz = stp.tile([4, 1], F32, tag=f"rz{i}")
                nc.vector.reciprocal(out=rz[:, :], in_=zs[:, :])
                atwn = stp.tile([4, N], F16, tag=f"atwn{i}")
                nc.vector.tensor_scalar_mul(out=atwn[:, :], in0=atw[:, :],
                                            scalar1=rz[:, :])
                return atwn

            def g_ctx(i, t, atwn):
                p12 = psp[i].tile([128, 2, 4], F16, tag="sc",
                                  name=f"tp{t}_{i}")
                nc.tensor.transpose(p12[:, 0, :], atwn[:, 0:128],
                                    id8[0:4, 0:4])
                nc.tensor.transpose(p12[0:N - 128, 1, :], atwn[:, 128:N],
                                    id8[0:4, 0:4])
                awt = stp.tile([128, 2, 4], F16, tag=f"awt{i}")
                nc.vector.tensor_copy(out=awt[:, 0, :], in_=p12[:, 0, :])
                nc.vector.tensor_copy(out=awt[0:N - 128, 1, :],
                                      in_=p12[0:N - 128, 1, :])
                cps = pdx[i].tile([128, 4, 4], F32, tag="dx",
                                  name=f"ctx{t}_{i}")
                bank_open(flat(cps, 16), 16, dep=atwn[0:1, 0:16])
                for b in range(4):
                    bb = 4 * i + b
                    for at in range(4):
                        asl = slice(at * 128, (at + 1) * 128)
                        nc.tensor.matmul(
                            cps[:, at, b:b + 1], enr[0:128, 2 * bb, asl],
                            awt[0:128, 0, b:b + 1],
                            start=False, stop=False)
                        nc.tensor.matmul(
                            cps[:, at, b:b + 1],
                            enr[0:N - 128, 2 * bb + 1, asl],
                            awt[0:N - 128, 1, b:b + 1],
                            start=False, stop=False)
                bank_close(flat(cps, 16), 16)
                ctxt = stp.tile([128, 4, 4], F16, tag=f"ctxt{i}")
                nc.vector.tensor_copy(out=ctxt[:, :, :], in_=cps[:, :, :])
                return ctxt

            def g_gates_ctx(i, gps, ctxt):
                for g in range(16):
                    gsl = slice(g * 128, (g + 1) * 128)
                    for k in range(4):
                        nc.tensor.matmul(gps[:, g, :], wc[:, k, gsl],
                                         ctxt[:, k, :],
                                         start=False, stop=False)
                bank_close(flat(gps, 64), 64)

            def g_pointwise(i, t, gps):
                csl = slice(4 * i, 4 * i + 4)
                th = stp.tile([128, 16, 4], F16, tag=f"th{i}")
                nc.scalar.activation(out=th[:, :, :], in_=gps[:, :, :],
                                     func=TANH, scale=0.5)
                eng = nc.gpsimd if i == 1 else nc.vector
                a2 = stp.tile([128, 4, 4], F32, tag=f"a2{i}")
                eng.scalar_tensor_tensor(
                    out=a2[:, :, :], in0=th[:, 4:8, :], scalar=1.0,
                    in1=c2[:, :, csl], op0=ADD, op1=MULT)
                bb = stp.tile([128, 4, 4], F32, tag=f"bb{i}")
                eng.scalar_tensor_tensor(
                    out=bb[:, :, :], in0=th[:, 0:4, :], scalar=1.0,
                    in1=th[:, 12:16, :], op0=ADD, op1=MULT)
                eng.scalar_tensor_tensor(
                    out=c2[:, :, csl], in0=a2[:, :, :], scalar=0.5,
                    in1=bb[:, :, :], op0=MULT, op1=ADD)
                thc = stp.tile([128, 4, 4], F16, tag=f"thc{i}")
                nc.scalar.activation(out=thc[:, :, :], in_=c2[:, :, csl],
                                     func=TANH, scale=0.5)
                nc.vector.scalar_tensor_tensor(
                    out=hallt[:, :, t * 8 + 4 * i:t * 8 + 4 * i + 4],
                    in0=th[:, 8:12, :],
                    scalar=1.0, in1=thc[:, :, :], op0=ADD, op1=MULT)

            for t in range(T):
                # dec + adds for both groups lead; gates-h fills PE while
                # the adds run on DVE/Pool
                dps = [None, None]
                gps = [None, None]
                xas = [None, None]
                for i in range(2):
                    if t > 0:
                        dps[i] = g_dec(i, t)
                    gps[i] = g_gates_h(i, t)
                    if t > 0:
                        xas[i] = g_adds(i, t, dps[i])
                # weave slot 1
                if t >= 16 and m0_jobs:
                    m0_mm(m0_jobs.pop(0))
                    m0_mm(m0_jobs.pop(0))
                if t < 4:
                    gx_pass(4 * t, 4 * t + 4, 40, 152)
                psc0 = g_tanh_scores(0, t, xas[0])
                if t >= 16:
                    m0_drain(nc.vector, limit=1)
                atwn0 = g_softmax(0, t, psc0)
                psc1 = g_tanh_scores(1, t, xas[1])
                ctxt0 = g_ctx(0, t, atwn0)
                g_gates_ctx(0, gps[0], ctxt0)
                atwn1 = g_softmax(1, t, psc1)
                # weave slot 2
                if t >= 16 and m0_jobs:
                    m0_mm(m0_jobs.pop(0))
                    m0_mm(m0_jobs.pop(0))
                ctxt1 = g_ctx(1, t, atwn1)
                g_pointwise(0, t, gps[0])
                g_gates_ctx(1, gps[1], ctxt1)
                if t >= 16:
                    m0_drain(nc.scalar, limit=1)
                    m0_drain(nc.vector, limit=1)
                g_pointwise(1, t, gps[1])
                if t >= 16 and m0_jobs:
                    m0_mm(m0_jobs.pop(0))
                    m0_mm(m0_jobs.pop(0))
                    m0_drain(nc.scalar, limit=1)
                    m0_drain(nc.vector, limit=1)

            m0_drain(nc.vector)

        # -------- tail: m1 transposed per group + m0 leftovers --------
        with tc.tile_pool(name="fom", bufs=3) as fom, \
             tc.tile_pool(name="pf2", bufs=2, space="PSUM") as pf2, \
             tc.tile_pool(name="pt1", bufs=2, space="PSUM") as pt1p:

            def m1t_group(g, eng):
                pt1 = pt1p.tile([VT, 16, 24], F32, tag="pt1", name=f"pt1{g}")
                bank_open(pt1[:, 0, 0:1], 1, npart=VT)
                for v in range(16):
                    for k in range(4):
                        nc.tensor.matmul(
                            pt1[:, v, :],
                            fcpre[:, k, g * 16 * VT + v * VT:
                                  g * 16 * VT + (v + 1) * VT],
                            hallt[:, k, 128:152],
                            start=False, stop=False)
                bank_close(pt1[:, 0, 0:1], 1, npart=VT)
                fo1 = fom.tile([VT, 16, 24], F16, tag="fo1", name=f"fo1{g}")
                if eng is nc.scalar:
                    eng.copy(out=fo1[:, :, :], in_=pt1[:, :, :])
                    dma = nc.sync
                else:
                    eng.tensor_copy(out=fo1[:, :, :], in_=pt1[:, :, :])
                    dma = nc.scalar
                dma.dma_start(
                    out=d_outT[g * 16 * VT:(g + 1) * 16 * VT, :].rearrange(
                        "(v p) c -> p v c", p=VT),
                    in_=fo1[:, :, :])

            def m0_chunk2(c0, eng):
                # leftover chunks in aligned consecutive pairs; 512-wide
                # slots keep each chunk's matmul output bank-aligned
                pf = pf2.tile([128, 2, 512], F32, tag="pf2", name=f"pfb{c0}")
                for cc in range(2):
                    for k in range(4):
                        nc.tensor.matmul(
                            pf[:, cc, 0:VC], hallt[:, k, 0:128],
                            fcpre[:, k, (c0 + cc) * VC:(c0 + cc + 1) * VC],
                            start=(k == 0), stop=(k == 3))
                fo = fom.tile([128, 2, VC], F16, tag="fo2", name=f"fob{c0}")
                if eng is nc.scalar:
                    eng.copy(out=fo[:, :, :], in_=pf[:, :, 0:VC])
                    dma = nc.sync
                else:
                    eng.tensor_copy(out=fo[:, :, :], in_=pf[:, :, 0:VC])
                    dma = nc.scalar
                dma.dma_start(
                    out=d_out[:, c0 * VC:(c0 + 2) * VC],
                    in_=fo[:, :, :].rearrange("p a b -> p (a b)"))

            assert len(m0_jobs) % 2 == 0 and m0_jobs == sorted(m0_jobs)
            pairs = [m0_jobs[j] for j in range(0, len(m0_jobs), 2)]
            engs = [nc.vector, nc.scalar]
            i = 0
            for g in range(NFG):
                m1t_group(g, engs[i % 2]); i += 1
                while pairs and len(pairs) > (NFG - 1 - g) * 14 // 10:
                    m0_chunk2(pairs.pop(0), engs[i % 2]); i += 1


_PROGRAM = None


def kernel(**inputs) -> np.ndarray:
    global _PROGRAM
    if _PROGRAM is None:
        _PROGRAM = build_program()
    in_maps = [prep_core(c, inputs) for c in range(NC)]
    res = run_bass_kernel_spmd(_PROGRAM, in_maps, core_ids=list(range(NC)))
    fcb = np.asarray(inputs["fc_b"], np.float32)
    out = np.zeros((B, L, V), np.float32)
    for c in range(NC):
        lg0 = res.results[c]["logits"].astype(np.float32) / FCS  # [128, V]
        lg0 = lg0.reshape(16, BS, V)
        out[c * BS:(c + 1) * BS, 1:17, :] = lg0.transpose(1, 0, 2) + fcb
        lgT = res.results[c]["logitsT"].astype(np.float32) / FCS  # [V, 24]
        lg1 = lgT.T.reshape(3, BS, V)
        out[c * BS:(c + 1) * BS, 17:, :] = lg1.transpose(1, 0, 2) + fcb
    return out


if __name__ == "__main__":
    import reference
    ins = {k: np.asarray(v) for k, v in reference.setup_inputs().items()}
    got = kernel(**ins)
    exp = np.asarray(reference.reference(**reference.setup_inputs()))
    err = np.abs(got - exp).max() / (np.abs(exp).max() + 1e-12)
    print("Relative error:", err)


# revision 34
# speedup vs baseline: 1.0209x; 1.0209x over previous
"""DecoderLSTM (Bahdanau attention + LSTM + vocab fc) on 8 Trainium2 cores.

Sharding: data-parallel over batch (64 -> 8 rows/core); the sequential scan
stays local per core; zero collectives. Host shards/casts/transposes inputs
and reassembles (f16 logits -> f32 + fc_b on host).

Feature-major recurrence: state h2T/c2T kept transposed [a-tiles, 8], so all
LSTM-side matmuls have n=8 (PE cost ~ output free size), and the pointwise
ops run on [128, 4, 8] tiles.

v1 changes over the 350us baseline:
- scores via fp8 DoubleRow matmuls: X = tanh(...) written as e4m3, diagonal
  energy-weight trick packed per at-pair [128,2,8]; exp gets scale=1/16 to
  undo the x16 pre-scale that keeps ew out of fp8 denormals.
- broadcast adds (ept + decT) as single DVE tensor_tensor ops reading the
  dec PSUM directly (at0-2) + one Pool op (at3); no per-b tensor_scalar.
- gates x-part precomputed once into gx (PE identity-matmul per step folds
  it into the PSUM accumulation) instead of 5 matmuls per g-tile per step.
- fc m0 (rows 0..127) in 2-chunk drain groups: one [128,1000] drain copy +
  one DMA each; fetch in 4-chunk [128,4,2000] group DMAs.
- fc m1 (rows 128..151) computed TRANSPOSED at the tail: out [125-vtile, 24]
  per group, 64 accumulating matmuls per bank, one [125,384] drain + DMA to
  d_outT; host re-transposes. Kills the 33us serial m-major tail.
- no gpsimd-issued DMAs (Pool engine was burning 1us per issue).
"""

import numpy as np
import ml_dtypes

import concourse.bass as bass
import concourse.bacc as bacc
import concourse.tile as tile
from concourse import mybir
from concourse.bass_utils import run_bass_kernel_spmd

F16 = mybir.dt.float16
F32 = mybir.dt.float32
F8 = mybir.dt.float8e4
F8E3 = mybir.dt.float8e3
E4M3 = ml_dtypes.float8_e4m3
E3M4 = ml_dtypes.float8_e3m4
DR = mybir.MatmulPerfMode.DoubleRow

B, N, H, E, A, V, L = 64, 196, 512, 512, 512, 20000, 20
T = L - 1            # 19 decode steps
NC = 8               # cores
BS = B // NC         # 8 batch rows per core
BN = BS * N          # 1568
BT = T * BS          # 152 rows, t-major (row = t*8 + b)
VC = 500             # fc vocab chunk width
NCH = V // VC        # 40
NFG = 10             # fetch groups (4 chunks each)
VT = 125             # m1-transposed vocab tile width (16 per fetch group)
EWS = 16.0           # energy_W pre-scale (undone by exp scale)
FCS = 32.0           # fc_W pre-scale into e3m4 normal range (undone on host)

# gate reorder [i,f,g,o] -> [i,f,o,g] so tanh(0.5*x) covers cols 0:1536
PERM = np.concatenate([np.arange(0, H), np.arange(H, 2 * H),
                       np.arange(3 * H, 4 * H), np.arange(2 * H, 3 * H)])

TANH = mybir.ActivationFunctionType.Tanh
EXP = mybir.ActivationFunctionType.Exp
ADD = mybir.AluOpType.add
MULT = mybir.AluOpType.mult


def prep_core(core, inputs):
    """Per-core numpy input dict (shard + transpose + cast only)."""
    f32 = np.float32
    bsl = slice(core * BS, (core + 1) * BS)
    enc = np.asarray(inputs["encoder_outputs"][bsl], f32)      # [8,196,512]

    enc_t = np.ascontiguousarray(enc.reshape(BN, H).T).astype(np.float16)
    enc_r = np.zeros((2 * BS, 128, H), np.float16)
    for b in range(BS):
        enc_r[2 * b, :128] = enc[b, :128]
        enc_r[2 * b + 1, :N - 128] = enc[b, 128:]

    caps = np.asarray(inputs["captions"][bsl])[:, :T]          # [8,19]
    es = np.asarray(inputs["emb"], f32)[caps]                  # [8,19,512]
    emb_flat = es.transpose(1, 0, 2).reshape(BT, E)            # t-major rows
    emb_cat = np.concatenate(
        [emb_flat.T, np.ones((1, BT), f32)], 0).astype(np.float16)

    wih = np.asarray(inputs["W_ih"], f32)[PERM]                # [2048,1024]
    whh = np.asarray(inputs["W_hh"], f32)[PERM]
    bias = (np.asarray(inputs["b_ih"], f32) +
            np.asarray(inputs["b_hh"], f32))[PERM]
    wihxb_t = np.concatenate(
        [wih[:, :E].T, bias[None, :]], 0)                      # [513,2048]
    wc_t = np.concatenate([wih[:, E:].T, 0.5 * whh.T], 0)      # [1024,2048]
    # pre-double the g-gate columns so the pointwise uses one tanh(0.5 x)
    wihxb_t[:, 3 * H:] *= 2.0
    wc_t[:, 3 * H:] *= 2.0
    wihxb_t = wihxb_t.astype(np.float16)
    wc_t = wc_t.astype(np.float16)

    dec_wt = (0.5 * np.asarray(inputs["dec_W"], f32).T).astype(np.float16)
    enc_wt = np.ascontiguousarray(
        np.asarray(inputs["enc_W"], f32).T).astype(np.float16)  # [H,A]
    ebdb = np.ascontiguousarray(
        (np.asarray(inputs["enc_b"], f32) +
         np.asarray(inputs["dec_b"], f32)).reshape(4, 128).T)   # [128,4]
    # diagonal energy-weight trick, fp8, at-pair packed, per b-group:
    # [128, 2ap, 2i, 2grp, 16]; nonzero at col bl4*4+bl4 (4x4 diagonal)
    ew8 = (EWS * np.asarray(inputs["energy_W"], f32)[0]).astype(E4M3)
    ewm8 = np.zeros((128, 2, 2, 2, 16), E4M3)
    for ap in range(2):
        for i in range(2):
            seg = ew8[ap * 256 + i * 128: ap * 256 + (i + 1) * 128]
            for g in range(2):
                ewm8[:, ap, i, g, 0:16:5] = np.repeat(seg[:, None], 4, 1)
    fcw_t = np.ascontiguousarray(
        0.5 * FCS * np.asarray(inputs["fc_W"], f32).T).astype(E3M4)
    id8 = np.eye(8, dtype=np.float16)
    id128 = np.eye(128, dtype=np.float16)

    return {"enc_t": enc_t, "enc_r": enc_r, "emb_cat": emb_cat,
            "wihxb_t": wihxb_t, "wc_t": wc_t, "dec_wt": dec_wt,
            "enc_wt": enc_wt, "ebdb": ebdb, "ewm8": ewm8,
            "fcw_t": fcw_t, "id8": id8, "id128": id128}


def _bcast(ap, n):
    """Append an innermost step-0 (broadcast) dim of size n to an AP."""
    return bass.AP(tensor=ap.tensor, offset=ap.offset,
                   ap=list(ap.ap) + [[0, n]])


def build_program():
    nc = bacc.Bacc("TRN2", target_bir_lowering=False, debug=False,
                   num_devices=NC)
    d_enc_t = nc.dram_tensor("enc_t", [H, BN], F16, kind="ExternalInput")
    d_enc_r = nc.dram_tensor("enc_r", [2 * BS, 128, H], F16,
                             kind="ExternalInput")
    d_emb = nc.dram_tensor("emb_cat", [E + 1, BT], F16, kind="ExternalInput")
    d_wx = nc.dram_tensor("wihxb_t", [E + 1, 4 * H], F16,
                          kind="ExternalInput")
    d_wc = nc.dram_tensor("wc_t", [2 * H, 4 * H], F16, kind="ExternalInput")
    d_dwt = nc.dram_tensor("dec_wt", [H, A], F16, kind="ExternalInput")
    d_ewt = nc.dram_tensor("enc_wt", [H, A], F16, kind="ExternalInput")
    d_ebdb = nc.dram_tensor("ebdb", [128, 4], F32, kind="ExternalInput")
    d_ewm8 = nc.dram_tensor("ewm8", [128, 2, 2, 2, 16], F8,
                            kind="ExternalInput")
    d_fcw = nc.dram_tensor("fcw_t", [H, V], F8E3, kind="ExternalInput")
    d_id8 = nc.dram_tensor("id8", [8, 8], F16, kind="ExternalInput")
    d_id128 = nc.dram_tensor("id128", [128, 128], F16, kind="ExternalInput")
    d_out = nc.dram_tensor("logits", [128, V], F16, kind="ExternalOutput")
    d_outT = nc.dram_tensor("logitsT", [V, 24], F16, kind="ExternalOutput")

    with tile.TileContext(nc) as tc:
        _build_body(nc, tc, d_enc_t, d_enc_r, d_emb, d_wx, d_wc, d_dwt,
                    d_ewt, d_ebdb, d_ewm8, d_fcw, d_id8, d_id128,
                    d_out, d_outT)
    nc.compile()
    return nc


def _build_body(nc, tc, d_enc_t, d_enc_r, d_emb, d_wx, d_wc, d_dwt, d_ewt,
                d_ebdb, d_ewm8, d_fcw, d_id8, d_id128, d_out, d_outT):
    with tc.tile_pool(name="res", bufs=1) as res:
        # -------- residents --------
        ept = res.tile([128, 4, BN], F16)        # enc_projT (+enc_b+dec_b)
        enr = res.tile([128, 2 * BS, H], F16)    # enc rows [n-tiles, h]
        wc = res.tile([128, 8, 4 * H], F16)      # [ctx;h] gate weights^T
        dwt = res.tile([128, 4, A], F16)         # 0.5 dec_W^T
        gx = res.tile([128, 16, BT], F16)        # precomputed x-part gates
        hallt = res.tile([128, 4, BT], F16)      # h2^T, all steps
        xt8 = res.tile([128, 4, BN], F8)         # tanh(X) in fp8 (scores rhs)
        ewm8 = res.tile([128, 2, 2, 2, 16], F8)  # diag ew, grouped
        id8 = res.tile([8, 8], F16)
        id128 = res.tile([128, 128], F16)
        c2 = res.tile([128, 4, 8], F32)          # 2c, transposed
        ebdb = res.tile([128, 4], F32)
        fcpre = res.tile([128, 4, V], F8E3)
        emb = res.tile([128, 5, BT], F16)        # emb_cat k-tiles (+ones row)
        wx = res.tile([128, 5, 4 * H], F16)      # W_ih_x^T (+bias row)

        # tiny first, then DMAs ordered by first use
        nc.sync.dma_start(out=id8[:, :], in_=d_id8[:, :])
        nc.sync.dma_start(out=ebdb[:, :], in_=d_ebdb[:, :])
        nc.sync.dma_start(out=ewm8[:, :, :, :, :],
                          in_=d_ewm8[:, :, :, :, :])
        nc.sync.dma_start(out=id128[:, :], in_=d_id128[:, :])
        nc.vector.memset(c2[:, :, :], 0.0)
        z1 = res.tile([1, 512], F16)
        nc.vector.memset(z1[:, :], 0.0)
        zf = res.tile([1, 512], F32)
        nc.vector.memset(zf[:, :], 0.0)
        # touch Tanh+Exp early so the one-time activation table load (1.3us)
        # happens during the setup DMAs, not on step 0's critical path
        warm = res.tile([1, 8], F16)
        nc.scalar.activation(out=warm[0:1, 0:8], in_=z1[0:1, 0:8], func=TANH)
        nc.scalar.activation(out=warm[0:1, 0:8], in_=z1[0:1, 0:8], func=EXP)

        def flat(tile_ap, ncols):
            return tile_ap.rearrange("p a b -> p (a b)")

        def bank_open(ap, ncols, dep=None, npart=128):
            # full-tile zero matmul: zeroes the 2KB bank and starts its single
            # accumulation group. lhsT is all-zero so the rhs (an optional
            # SBUF tile produced by the op this must execute after) only
            # carries a read dependency.
            rhs = z1[0:1, 0:ncols] if dep is None else dep
            lhs = zf if rhs.dtype == F32 else z1
            nc.tensor.matmul(ap, lhs[0:1, 0:npart], rhs,
                             start=True, stop=False)

        def bank_close(ap, ncols, dep=None, npart=128):
            rhs = z1[0:1, 0:ncols] if dep is None else dep
            lhs = zf if rhs.dtype == F32 else z1
            nc.tensor.matmul(ap, lhs[0:1, 0:npart], rhs,
                             start=False, stop=True)

        # -------- setup: enc_projT (+enc_b+dec_b), gates-x precompute ------
        with tc.tile_pool(name="se", bufs=1) as se, \
             tc.tile_pool(name="sep", bufs=2, space="PSUM") as sep:
            et = se.tile([128, 4, BN], F16)
            ewt = se.tile([128, 4, A], F16)
            nc.sync.dma_start(
                out=ewt[:, :, :],
                in_=d_ewt[:, :].rearrange("(k p) a -> p k a", p=128))
            for ch in range(4):                      # chunked: MMs start early
                nc.sync.dma_start(
                    out=et[:, :, ch * 392:(ch + 1) * 392],
                    in_=d_enc_t[:, ch * 392:(ch + 1) * 392].rearrange(
                        "(k p) n -> p k n", p=128))
            nc.sync.dma_start(
                out=dwt[:, :, :],
                in_=d_dwt[:, :].rearrange("(k p) a -> p k a", p=128))
            nc.sync.dma_start(
                out=emb[:, 0:4, :],
                in_=d_emb[0:512, :].rearrange("(k p) t -> p k t", p=128))
            nc.sync.dma_start(out=emb[0:1, 4, :], in_=d_emb[512:513, :])
            nc.sync.dma_start(
                out=wx[:, 0:4, :],
                in_=d_wx[0:512, :].rearrange("(k p) g -> p k g", p=128))
            nc.sync.dma_start(out=wx[0:1, 4, :], in_=d_wx[512:513, :])
            nc.sync.dma_start(out=enr[:, :, :],
                              in_=d_enc_r[:, :, :].rearrange("j p h -> p j h"))
            # wc: ctx half (k 0:4) first (needed at t0), then h half
            nc.sync.dma_start(
                out=wc[:, 0:4, :],
                in_=d_wc[0:512, :].rearrange("(k p) g -> p k g", p=128))
            nc.sync.dma_start(
                out=wc[:, 4:8, :],
                in_=d_wc[512:1024, :].rearrange("(k p) g -> p k g", p=128))
            for q in range(4):                    # whole fc_W, fp8 resident
                nc.sync.dma_start(
                    out=fcpre[:, :, q * 5000:(q + 1) * 5000],
                    in_=d_fcw[:, q * 5000:(q + 1) * 5000].rearrange(
                        "(k p) v -> p k v", p=128))
            for at in range(4):                      # a-tile = out m-tile
                for ch in range(4):                  # 1568 = 4*392
                    pt = sep.tile([128, 392], F32, tag="sep")
                    for k in range(4):
                        nc.tensor.matmul(
                            pt[:, :],
                            ewt[:, k, at * 128:(at + 1) * 128],
                            et[:, k, ch * 392:(ch + 1) * 392],
                            start=(k == 0), stop=(k == 3))
                    # alternate the bias-add drain across DVE and ACT so
                    # neither engine serializes the whole ept production
                    osl = ept[:, at, ch * 392:(ch + 1) * 392]
                    if ch % 2 == 0:
                        nc.vector.tensor_scalar_add(
                            out=osl, in0=pt[:, :],
                            scalar1=ebdb[:, at:at + 1])
                    else:
                        nc.scalar.activation(
                            out=osl, in_=pt[:, :],
                            func=mybir.ActivationFunctionType.Identity,
                            bias=ebdb[:, at:at + 1])
            # gates-x: gx[:, g, :] = (wihxb^T emb_cat); only the first 5
            # steps' columns here -- the rest weaves into steps 0-1
            for g in range(16):
                gsl = slice(g * 128, (g + 1) * 128)
                ptg = sep.tile([128, 40], F32, tag="sep")
                for k in range(5):
                    kr = 128 if k < 4 else 1
                    nc.tensor.matmul(ptg[:, :], wx[0:kr, k, gsl],
                                     emb[0:kr, k, 0:40],
                                     start=(k == 0), stop=(k == 4))
                nc.vector.tensor_copy(out=gx[:, g, 0:40], in_=ptg[:, :])

        # -------- recurrence, 2-way b-split software pipeline --------
        # Groups G0 = b 0..3, G1 = b 4..7 run their serial attention/LSTM
        # chains staggered so ACT's tanh burst for one group overlaps the
        # other group's softmax/ctx/gates glue. fc m0 chunks weave into PE
        # idle at t>=16 (1-bank jobs, 2 PSUM bufs ping-pong).
        m0_jobs = list(range(NCH))

        with tc.tile_pool(name="pdx0", bufs=1, space="PSUM") as pdx0, \
             tc.tile_pool(name="pdx1", bufs=1, space="PSUM") as pdx1, \
             tc.tile_pool(name="pg0", bufs=1, space="PSUM") as pg0, \
             tc.tile_pool(name="pg1", bufs=1, space="PSUM") as pg1, \
             tc.tile_pool(name="ps0", bufs=1, space="PSUM") as ps0, \
             tc.tile_pool(name="ps1", bufs=1, space="PSUM") as ps1, \
             tc.tile_pool(name="pf", bufs=2, space="PSUM") as pfp, \
             tc.tile_pool(name="stp", bufs=2) as stp, \
             tc.tile_pool(name="fo", bufs=3) as fop, \
             tc.tile_pool(name="xp", bufs=1) as xp:

            pdx = [pdx0, pdx1]
            pgt = [pg0, pg1]
            psp = [ps0, ps1]
            fc_pending = []

            def m0_mm(ch):
                pf = pfp.tile([128, VC], F32, tag="pf", name=f"pf{ch}")
                for k in range(4):
                    nc.tensor.matmul(
                        pf[:, :], hallt[:, k, 0:128],
                        fcpre[:, k, ch * VC:(ch + 1) * VC],
                        start=(k == 0), stop=(k == 3))
                fc_pending.append((pf, ch))

            def m0_drain(eng, limit=None):
                nd = len(fc_pending) if limit is None else \
                    min(limit, len(fc_pending))
                for _ in range(nd):
                    pf, ch = fc_pending.pop(0)
                    fo = fop.tile([128, VC], F16, tag="fo", name=f"fo{ch}")
                    if eng is nc.scalar:
                        eng.copy(out=fo[:, :], in_=pf[:, :])
                    else:
                        eng.tensor_copy(out=fo[:, :], in_=pf[:, :])
                    nc.sync.dma_start(
                        out=d_out[:, ch * VC:(ch + 1) * VC], in_=fo[:, :])

            def gx_pass(g0, g1, c0, c1):
                for g in range(g0, g1):
                    gsl = slice(g * 128, (g + 1) * 128)
                    ptg = pfp.tile([128, c1 - c0], F32, tag="pf",
                                   name=f"gxb{g}_{c0}")
                    for k in range(5):
                        kr = 128 if k < 4 else 1
                        nc.tensor.matmul(ptg[:, :], wx[0:kr, k, gsl],
                                         emb[0:kr, k, c0:c1],
                                         start=(k == 0), stop=(k == 4))
                    nc.vector.tensor_copy(out=gx[:, g, c0:c1], in_=ptg[:, :])

            GW = 4 * N  # 784: one b-group's column width in (b n) layouts

            def g_dec(i, t):
                """dec matmuls for group i -> dps_i [128, 4at, 4b]."""
                dps = pdx[i].tile([128, 4, 4], F32, tag="dx",
                                  name=f"dec{t}_{i}")
                hsl = slice(t * 8 + 4 * i - 8, t * 8 + 4 * i - 4)
                bank_open(flat(dps, 16), 16,
                          dep=hallt[0:1, :, hsl])
                for at in range(4):
                    for k in range(4):
                        nc.tensor.matmul(
                            dps[:, at, :],
                            dwt[:, k, at * 128:(at + 1) * 128],
                            hallt[:, k, hsl], start=False, stop=False)
                bank_close(flat(dps, 16), 16)
                return dps

            def g_gates_h(i, t):
                gps = pgt[i].tile([128, 16, 4], F32, tag="g",
                                  name=f"g{t}_{i}")
                hsl = slice(t * 8 + 4 * i - 8, t * 8 + 4 * i - 4)
                xsl = slice(t * 8 + 4 * i, t * 8 + 4 * i + 4)
                bank_open(flat(gps, 64), 64)
                for g in range(16):
                    gsl = slice(g * 128, (g + 1) * 128)
                    if t > 0:
                        for k in range(4):
                            nc.tensor.matmul(gps[:, g, :], wc[:, 4 + k, gsl],
                                             hallt[:, k, hsl],
                                             start=False, stop=False)
                    nc.tensor.matmul(gps[:, g, :], id128[:, :],
                                     gx[:, g, xsl], start=False, stop=False)
                return gps

            def g_adds(i, t, dps):
                """ept + dec broadcast: DVE at0/at2 (PSUM direct), Pool
                at1/at3 (via dect copies)."""
                dect = stp.tile([128, 2, 4], F32, tag=f"dect{i}",
                                name=f"dect{t}_{i}")
                nc.vector.tensor_copy(out=dect[:, 0, :], in_=dps[:, 1, :])
                nc.vector.tensor_copy(out=dect[:, 1, :], in_=dps[:, 3, :])
                xas = []
                for ap in range(2):
                    xa = xp.tile([128, 2, GW], F16, tag=f"x{i}_{ap}",
                                 name=f"xa{t}_{i}{ap}")
                    xas.append(xa)
                for at in (1, 0, 3, 2):
                    ap, j = at // 2, at % 2
                    src = ept[:, at, i * GW:(i + 1) * GW].rearrange(
                        "p (b n) -> p b n", n=N)
                    dst = xas[ap][:, j, :].rearrange("p (b n) -> p b n", n=N)
                    if j == 0:
                        nc.vector.tensor_add(
                            out=dst, in0=src, in1=_bcast(dps[:, at, :], N))
                    else:
                        nc.gpsimd.tensor_add(
                            out=dst, in0=src,
                            in1=_bcast(dect[:, at // 2, :], N))
                return xas

            def g_tanh_scores(i, t, xas):
                psc = psp[i].tile([4, N], F32, tag="sc", name=f"sc{t}_{i}")
                for ap in range(2):
                    if t == 0:
                        nc.scalar.activation(
                            out=xt8[:, 2 * ap:2 * ap + 2,
                                    i * GW:(i + 1) * GW],
                            in_=ept[:, 2 * ap:2 * ap + 2,
                                    i * GW:(i + 1) * GW], func=TANH)
                    else:
                        nc.scalar.activation(
                            out=xt8[:, 2 * ap:2 * ap + 2,
                                    i * GW:(i + 1) * GW],
                            in_=xas[ap][:, :, :], func=TANH)
                for ap in range(2):
                    for bl in range(4):
                        nc.tensor.matmul(
                            psc[:, :],
                            ewm8[:, ap, :, i, bl * 4:(bl + 1) * 4],
                            xt8[:, 2 * ap:2 * ap + 2,
                                i * GW + bl * N:i * GW + (bl + 1) * N],
                            start=(ap == 0 and bl == 0),
                            stop=(ap == 1 and bl == 3),
                            perf_mode=DR)
                return psc

            def g_softmax(i, t, psc):
                atw = stp.tile([4, N], F16, tag=f"atw{i}")
                zs = stp.tile([4, 1], F32, tag=f"zs{i}")
                nc.scalar.activation(out=atw[:, :], in_=psc[:, :],
                                     func=EXP, scale=1.0 / EWS,
                                     accum_out=zs[:, 0:1])
                rz = stp.tile([4, 1], F32, tag=f"rz{i}")
                nc.vector.reciprocal(out=rz[:, :], in_=zs[:, :])
                atwn = stp.tile([4, N], F16, tag=f"atwn{i}")
                nc.vector.tensor_scalar_mul(out=atwn[:, :], in0=atw[:, :],
                                            scalar1=rz[:, :])
                return atwn

            def g_ctx(i, t, atwn):
                p12 = psp[i].tile([128, 2, 4], F16, tag="sc",
                                  name=f"tp{t}_{i}")
                nc.tensor.transpose(p12[:, 0, :], atwn[:, 0:128],
                                    id8[0:4, 0:4])
                nc.tensor.transpose(p12[0:N - 128, 1, :], atwn[:, 128:N],
                                    id8[0:4, 0:4])
                awt = stp.tile([128, 2, 4], F16, tag=f"awt{i}")
                nc.vector.tensor_copy(out=awt[:, 0, :], in_=p12[:, 0, :])
                nc.vector.tensor_copy(out=awt[0:N - 128, 1, :],
                                      in_=p12[0:N - 128, 1, :])
                cps = pdx[i].tile([128, 4, 4], F32, tag="dx",
                                  name=f"ctx{t}_{i}")
                bank_open(flat(cps, 16), 16, dep=atwn[0:1, 0:16])
                for b in range(4):
                    bb = 4 * i + b
                    for at in range(4):
                        asl = slice(at * 128, (at + 1) * 128)
                        nc.tensor.matmul(
                            cps[:, at, b:b + 1], enr[0:128, 2 * bb, asl],
                            awt[0:128, 0, b:b + 1],
                            start=False, stop=False)
                        nc.tensor.matmul(
                            cps[:, at, b:b + 1],
                            enr[0:N - 128, 2 * bb + 1, asl],
                            awt[0:N - 128, 1, b:b + 1],
                            start=False, stop=False)
                bank_close(flat(cps, 16), 16)
                ctxt = stp.tile([128, 4, 4], F16, tag=f"ctxt{i}")
                nc.vector.tensor_copy(out=ctxt[:, :, :], in_=cps[:, :, :])
                return ctxt

            def g_gates_ctx(i, gps, ctxt):
                for g in range(16):
                    gsl = slice(g * 128, (g + 1) * 128)
                    for k in range(4):
                        nc.tensor.matmul(gps[:, g, :], wc[:, k, gsl],
                                         ctxt[:, k, :],
                                         start=False, stop=False)
                bank_close(flat(gps, 64), 64)

            def g_pointwise(i, t, gps):
                csl = slice(4 * i, 4 * i + 4)
                th = stp.tile([128, 16, 4], F16, tag=f"th{i}")
                nc.scalar.activation(out=th[:, :, :], in_=gps[:, :, :],
                                     func=TANH, scale=0.5)
                a2 = stp.tile([128, 4, 4], F32, tag=f"a2{i}")
                nc.vector.scalar_tensor_tensor(
                    out=a2[:, :, :], in0=th[:, 4:8, :], scalar=1.0,
                    in1=c2[:, :, csl], op0=ADD, op1=MULT)
                bb = stp.tile([128, 4, 4], F32, tag=f"bb{i}")
                nc.vector.scalar_tensor_tensor(
                    out=bb[:, :, :], in0=th[:, 0:4, :], scalar=1.0,
                    in1=th[:, 12:16, :], op0=ADD, op1=MULT)
                nc.vector.scalar_tensor_tensor(
                    out=c2[:, :, csl], in0=a2[:, :, :], scalar=0.5,
                    in1=bb[:, :, :], op0=MULT, op1=ADD)
                thc = stp.tile([128, 4, 4], F16, tag=f"thc{i}")
                nc.scalar.activation(out=thc[:, :, :], in_=c2[:, :, csl],
                                     func=TANH, scale=0.5)
                nc.vector.scalar_tensor_tensor(
                    out=hallt[:, :, t * 8 + 4 * i:t * 8 + 4 * i + 4],
                    in0=th[:, 8:12, :],
                    scalar=1.0, in1=thc[:, :, :], op0=ADD, op1=MULT)

            for t in range(T):
                # dec + adds for both groups lead; gates-h fills PE while
                # the adds run on DVE/Pool
                dps = [None, None]
                gps = [None, None]
                xas = [None, None]
                for i in range(2):
                    if t > 0:
                        dps[i] = g_dec(i, t)
                    gps[i] = g_gates_h(i, t)
                    if t > 0:
                        xas[i] = g_adds(i, t, dps[i])
                # weave slot 1
                if t >= 16 and m0_jobs:
                    m0_mm(m0_jobs.pop(0))
                    m0_mm(m0_jobs.pop(0))
                if t < 4:
                    gx_pass(4 * t, 4 * t + 4, 40, 152)
                psc0 = g_tanh_scores(0, t, xas[0])
                if t >= 16:
                    m0_drain(nc.vector, limit=1)
                atwn0 = g_softmax(0, t, psc0)
                psc1 = g_tanh_scores(1, t, xas[1])
                ctxt0 = g_ctx(0, t, atwn0)
                g_gates_ctx(0, gps[0], ctxt0)
                atwn1 = g_softmax(1, t, psc1)
                # weave slot 2
                if t >= 16 and m0_jobs:
                    m0_mm(m0_jobs.pop(0))
                    m0_mm(m0_jobs.pop(0))
                ctxt1 = g_ctx(1, t, atwn1)
                g_pointwise(0, t, gps[0])
                g_gates_ctx(1, gps[1], ctxt1)
                if t >= 16:
                    m0_drain(nc.scalar, limit=1)
                    m0_drain(nc.vector, limit=1)
                g_pointwise(1, t, gps[1])
                if t >= 16:
                    m0_drain(nc.scalar, limit=1)
                    m0_drain(nc.vector, limit=1)

            m0_drain(nc.vector)

        # -------- tail: m1 transposed per group + m0 leftovers --------
        with tc.tile_pool(name="fom", bufs=3) as fom, \
             tc.tile_pool(name="pf2", bufs=2, space="PSUM") as pf2, \
             tc.tile_pool(name="pt1", bufs=2, space="PSUM") as pt1p:

            def m1t_group(g, eng):
                pt1 = pt1p.tile([VT, 16, 24], F32, tag="pt1", name=f"pt1{g}")
                bank_open(pt1[:, 0, 0:1], 1, npart=VT)
                for v in range(16):
                    for k in range(4):
                        nc.tensor.matmul(
                            pt1[:, v, :],
                            fcpre[:, k, g * 16 * VT + v * VT:
                                  g * 16 * VT + (v + 1) * VT],
                            hallt[:, k, 128:152],
                            start=False, stop=False)
                bank_close(pt1[:, 0, 0:1], 1, npart=VT)
                fo1 = fom.tile([VT, 16, 24], F16, tag="fo1", name=f"fo1{g}")
                if eng is nc.scalar:
                    eng.copy(out=fo1[:, :, :], in_=pt1[:, :, :])
                    dma = nc.sync
                else:
                    eng.tensor_copy(out=fo1[:, :, :], in_=pt1[:, :, :])
                    dma = nc.scalar
                dma.dma_start(
                    out=d_outT[g * 16 * VT:(g + 1) * 16 * VT, :].rearrange(
                        "(v p) c -> p v c", p=VT),
                    in_=fo1[:, :, :])

            def m0_chunk2(c0, eng):
                # leftover chunks in aligned consecutive pairs; 512-wide
                # slots keep each chunk's matmul output bank-aligned
                pf = pf2.tile([128, 2, 512], F32, tag="pf2", name=f"pfb{c0}")
                for cc in range(2):
                    for k in range(4):
                        nc.tensor.matmul(
                            pf[:, cc, 0:VC], hallt[:, k, 0:128],
                            fcpre[:, k, (c0 + cc) * VC:(c0 + cc + 1) * VC],
                            start=(k == 0), stop=(k == 3))
                fo = fom.tile([128, 2, VC], F16, tag="fo2", name=f"fob{c0}")
                if eng is nc.scalar:
                    eng.copy(out=fo[:, :, :], in_=pf[:, :, 0:VC])
                    dma = nc.sync
                else:
                    eng.tensor_copy(out=fo[:, :, :], in_=pf[:, :, 0:VC])
                    dma = nc.scalar
                dma.dma_start(
                    out=d_out[:, c0 * VC:(c0 + 2) * VC],
                    in_=fo[:, :, :].rearrange("p a b -> p (a b)"))

            assert len(m0_jobs) % 2 == 0 and m0_jobs == sorted(m0_jobs)
            pairs = [m0_jobs[j] for j in range(0, len(m0_jobs), 2)]
            engs = [nc.vector, nc.scalar]
            i = 0
            for g in range(NFG):
                m1t_group(g, engs[i % 2]); i += 1
                while pairs and len(pairs) > (NFG - 1 - g) * 14 // 10:
                    m0_chunk2(pairs.pop(0), engs[i % 2]); i += 1


_PROGRAM = None


def kernel(**inputs) -> np.ndarray:
    global _PROGRAM
    if _PROGRAM is None:
        _PROGRAM = build_program()
    in_maps = [prep_core(c, inputs) for c in range(NC)]
    res = run_bass_kernel_spmd(_PROGRAM, in_maps, core_ids=list(range(NC)))
    fcb = np.asarray(inputs["fc_b"], np.float32)
    out = np.zeros((B, L, V), np.float32)
    for c in range(NC):
        lg0 = res.results[c]["logits"].astype(np.float32) / FCS  # [128, V]
        lg0 = lg0.reshape(16, BS, V)
        out[c * BS:(c + 1) * BS, 1:17, :] = lg0.transpose(1, 0, 2) + fcb
        lgT = res.results[c]["logitsT"].astype(np.float32) / FCS  # [V, 24]
        lg1 = lgT.T.reshape(3, BS, V)
        out[c * BS:(c + 1) * BS, 17:, :] = lg1.transpose(1, 0, 2) + fcb
    return out


if __name__ == "__main__":
    import reference
    ins = {k: np.asarray(v) for k, v in reference.setup_inputs().items()}
    got = kernel(**ins)
    exp = np.asarray(reference.reference(**reference.setup_inputs()))
    err = np.abs(got - exp).max() / (np.abs(exp).max() + 1e-12)
    print("Relative error:", err)


# revision 35
# speedup vs baseline: 1.0434x; 1.0220x over previous
"""DecoderLSTM (Bahdanau attention + LSTM + vocab fc) on 8 Trainium2 cores.

Sharding: data-parallel over batch (64 -> 8 rows/core); the sequential scan
stays local per core; zero collectives. Host shards/casts/transposes inputs
and reassembles (f16 logits -> f32 + fc_b on host).

Feature-major recurrence: state h2T/c2T kept transposed [a-tiles, 8], so all
LSTM-side matmuls have n=8 (PE cost ~ output free size), and the pointwise
ops run on [128, 4, 8] tiles.

v1 changes over the 350us baseline:
- scores via fp8 DoubleRow matmuls: X = tanh(...) written as e4m3, diagonal
  energy-weight trick packed per at-pair [128,2,8]; exp gets scale=1/16 to
  undo the x16 pre-scale that keeps ew out of fp8 denormals.
- broadcast adds (ept + decT) as single DVE tensor_tensor ops reading the
  dec PSUM directly (at0-2) + one Pool op (at3); no per-b tensor_scalar.
- gates x-part precomputed once into gx (PE identity-matmul per step folds
  it into the PSUM accumulation) instead of 5 matmuls per g-tile per step.
- fc m0 (rows 0..127) in 2-chunk drain groups: one [128,1000] drain copy +
  one DMA each; fetch in 4-chunk [128,4,2000] group DMAs.
- fc m1 (rows 128..151) computed TRANSPOSED at the tail: out [125-vtile, 24]
  per group, 64 accumulating matmuls per bank, one [125,384] drain + DMA to
  d_outT; host re-transposes. Kills the 33us serial m-major tail.
- no gpsimd-issued DMAs (Pool engine was burning 1us per issue).
"""

import numpy as np
import ml_dtypes

import concourse.bass as bass
import concourse.bacc as bacc
import concourse.tile as tile
from concourse import mybir
from concourse.bass_utils import run_bass_kernel_spmd

F16 = mybir.dt.float16
F32 = mybir.dt.float32
F8 = mybir.dt.float8e4
F8E3 = mybir.dt.float8e3
E4M3 = ml_dtypes.float8_e4m3
E3M4 = ml_dtypes.float8_e3m4
DR = mybir.MatmulPerfMode.DoubleRow

B, N, H, E, A, V, L = 64, 196, 512, 512, 512, 20000, 20
T = L - 1            # 19 decode steps
NC = 8               # cores
BS = B // NC         # 8 batch rows per core
BN = BS * N          # 1568
BT = T * BS          # 152 rows, t-major (row = t*8 + b)
VC = 500             # fc vocab chunk width
NCH = V // VC        # 40
NFG = 10             # fetch groups (4 chunks each)
VT = 125             # m1-transposed vocab tile width (16 per fetch group)
EWS = 16.0           # energy_W pre-scale (undone by exp scale)
FCS = 32.0           # fc_W pre-scale into e3m4 normal range (undone on host)

# gate reorder [i,f,g,o] -> [i,f,o,g] so tanh(0.5*x) covers cols 0:1536
PERM = np.concatenate([np.arange(0, H), np.arange(H, 2 * H),
                       np.arange(3 * H, 4 * H), np.arange(2 * H, 3 * H)])

TANH = mybir.ActivationFunctionType.Tanh
EXP = mybir.ActivationFunctionType.Exp
ADD = mybir.AluOpType.add
MULT = mybir.AluOpType.mult


def prep_core(core, inputs):
    """Per-core numpy input dict (shard + transpose + cast only)."""
    f32 = np.float32
    bsl = slice(core * BS, (core + 1) * BS)
    enc = np.asarray(inputs["encoder_outputs"][bsl], f32)      # [8,196,512]

    enc_t = np.ascontiguousarray(enc.reshape(BN, H).T).astype(np.float16)
    enc_r = np.zeros((2 * BS, 128, H), np.float16)
    for b in range(BS):
        enc_r[2 * b, :128] = enc[b, :128]
        enc_r[2 * b + 1, :N - 128] = enc[b, 128:]

    caps = np.asarray(inputs["captions"][bsl])[:, :T]          # [8,19]
    es = np.asarray(inputs["emb"], f32)[caps]                  # [8,19,512]
    emb_flat = es.transpose(1, 0, 2).reshape(BT, E)            # t-major rows
    emb_cat = np.concatenate(
        [emb_flat.T, np.ones((1, BT), f32)], 0).astype(np.float16)

    wih = np.asarray(inputs["W_ih"], f32)[PERM]                # [2048,1024]
    whh = np.asarray(inputs["W_hh"], f32)[PERM]
    bias = (np.asarray(inputs["b_ih"], f32) +
            np.asarray(inputs["b_hh"], f32))[PERM]
    wihxb_t = np.concatenate(
        [wih[:, :E].T, bias[None, :]], 0)                      # [513,2048]
    wc_t = np.concatenate([wih[:, E:].T, 0.5 * whh.T], 0)      # [1024,2048]
    # pre-double the g-gate columns so the pointwise uses one tanh(0.5 x)
    wihxb_t[:, 3 * H:] *= 2.0
    wc_t[:, 3 * H:] *= 2.0
    wihxb_t = wihxb_t.astype(np.float16)
    wc_t = wc_t.astype(np.float16)

    dec_wt = (0.5 * np.asarray(inputs["dec_W"], f32).T).astype(np.float16)
    enc_wt = np.ascontiguousarray(
        np.asarray(inputs["enc_W"], f32).T).astype(np.float16)  # [H,A]
    ebdb = np.ascontiguousarray(
        (np.asarray(inputs["enc_b"], f32) +
         np.asarray(inputs["dec_b"], f32)).reshape(4, 128).T)   # [128,4]
    # diagonal energy-weight trick, fp8, at-pair packed, per b-group:
    # [128, 2ap, 2i, 2grp, 16]; nonzero at col bl4*4+bl4 (4x4 diagonal)
    ew8 = (EWS * np.asarray(inputs["energy_W"], f32)[0]).astype(E4M3)
    ewm8 = np.zeros((128, 2, 2, 2, 16), E4M3)
    for ap in range(2):
        for i in range(2):
            seg = ew8[ap * 256 + i * 128: ap * 256 + (i + 1) * 128]
            for g in range(2):
                ewm8[:, ap, i, g, 0:16:5] = np.repeat(seg[:, None], 4, 1)
    fcw_t = np.ascontiguousarray(
        0.5 * FCS * np.asarray(inputs["fc_W"], f32).T).astype(E3M4)
    id8 = np.eye(8, dtype=np.float16)
    id128 = np.eye(128, dtype=np.float16)

    return {"enc_t": enc_t, "enc_r": enc_r, "emb_cat": emb_cat,
            "wihxb_t": wihxb_t, "wc_t": wc_t, "dec_wt": dec_wt,
            "enc_wt": enc_wt, "ebdb": ebdb, "ewm8": ewm8,
            "fcw_t": fcw_t, "id8": id8, "id128": id128}


def _bcast(ap, n):
    """Append an innermost step-0 (broadcast) dim of size n to an AP."""
    return bass.AP(tensor=ap.tensor, offset=ap.offset,
                   ap=list(ap.ap) + [[0, n]])


def build_program():
    nc = bacc.Bacc("TRN2", target_bir_lowering=False, debug=False,
                   num_devices=NC)
    d_enc_t = nc.dram_tensor("enc_t", [H, BN], F16, kind="ExternalInput")
    d_enc_r = nc.dram_tensor("enc_r", [2 * BS, 128, H], F16,
                             kind="ExternalInput")
    d_emb = nc.dram_tensor("emb_cat", [E + 1, BT], F16, kind="ExternalInput")
    d_wx = nc.dram_tensor("wihxb_t", [E + 1, 4 * H], F16,
                          kind="ExternalInput")
    d_wc = nc.dram_tensor("wc_t", [2 * H, 4 * H], F16, kind="ExternalInput")
    d_dwt = nc.dram_tensor("dec_wt", [H, A], F16, kind="ExternalInput")
    d_ewt = nc.dram_tensor("enc_wt", [H, A], F16, kind="ExternalInput")
    d_ebdb = nc.dram_tensor("ebdb", [128, 4], F32, kind="ExternalInput")
    d_ewm8 = nc.dram_tensor("ewm8", [128, 2, 2, 2, 16], F8,
                            kind="ExternalInput")
    d_fcw = nc.dram_tensor("fcw_t", [H, V], F8E3, kind="ExternalInput")
    d_id8 = nc.dram_tensor("id8", [8, 8], F16, kind="ExternalInput")
    d_id128 = nc.dram_tensor("id128", [128, 128], F16, kind="ExternalInput")
    d_out = nc.dram_tensor("logits", [128, V], F16, kind="ExternalOutput")
    d_outT = nc.dram_tensor("logitsT", [V, 24], F16, kind="ExternalOutput")

    with tile.TileContext(nc) as tc:
        _build_body(nc, tc, d_enc_t, d_enc_r, d_emb, d_wx, d_wc, d_dwt,
                    d_ewt, d_ebdb, d_ewm8, d_fcw, d_id8, d_id128,
                    d_out, d_outT)
    nc.compile()
    return nc


def _build_body(nc, tc, d_enc_t, d_enc_r, d_emb, d_wx, d_wc, d_dwt, d_ewt,
                d_ebdb, d_ewm8, d_fcw, d_id8, d_id128, d_out, d_outT):
    with tc.tile_pool(name="res", bufs=1) as res:
        # -------- residents --------
        ept = res.tile([128, 4, BN], F16)        # enc_projT (+enc_b+dec_b)
        enr = res.tile([128, 2 * BS, H], F16)    # enc rows [n-tiles, h]
        wc = res.tile([128, 8, 4 * H], F16)      # [ctx;h] gate weights^T
        dwt = res.tile([128, 4, A], F16)         # 0.5 dec_W^T
        gx = res.tile([128, 16, BT], F16)        # precomputed x-part gates
        hallt = res.tile([128, 4, BT], F16)      # h2^T, all steps
        xt8 = res.tile([128, 4, BN], F8)         # tanh(X) in fp8 (scores rhs)
        ewm8 = res.tile([128, 2, 2, 2, 16], F8)  # diag ew, grouped
        id8 = res.tile([8, 8], F16)
        id128 = res.tile([128, 128], F16)
        c2 = res.tile([128, 4, 8], F32)          # 2c, transposed
        ebdb = res.tile([128, 4], F32)
        fcpre = res.tile([128, 4, V], F8E3)
        emb = res.tile([128, 5, BT], F16)        # emb_cat k-tiles (+ones row)
        wx = res.tile([128, 5, 4 * H], F16)      # W_ih_x^T (+bias row)

        # tiny first, then DMAs ordered by first use
        nc.sync.dma_start(out=id8[:, :], in_=d_id8[:, :])
        nc.sync.dma_start(out=ebdb[:, :], in_=d_ebdb[:, :])
        nc.sync.dma_start(out=ewm8[:, :, :, :, :],
                          in_=d_ewm8[:, :, :, :, :])
        nc.sync.dma_start(out=id128[:, :], in_=d_id128[:, :])
        nc.vector.memset(c2[:, :, :], 0.0)
        z1 = res.tile([1, 512], F16)
        nc.vector.memset(z1[:, :], 0.0)
        zf = res.tile([1, 512], F32)
        nc.vector.memset(zf[:, :], 0.0)
        # touch Tanh+Exp early so the one-time activation table load (1.3us)
        # happens during the setup DMAs, not on step 0's critical path
        warm = res.tile([1, 8], F16)
        nc.scalar.activation(out=warm[0:1, 0:8], in_=z1[0:1, 0:8], func=TANH)
        nc.scalar.activation(out=warm[0:1, 0:8], in_=z1[0:1, 0:8], func=EXP)

        def flat(tile_ap, ncols):
            return tile_ap.rearrange("p a b -> p (a b)")

        def bank_open(ap, ncols, dep=None, npart=128):
            # full-tile zero matmul: zeroes the 2KB bank and starts its single
            # accumulation group. lhsT is all-zero so the rhs (an optional
            # SBUF tile produced by the op this must execute after) only
            # carries a read dependency.
            rhs = z1[0:1, 0:ncols] if dep is None else dep
            lhs = zf if rhs.dtype == F32 else z1
            nc.tensor.matmul(ap, lhs[0:1, 0:npart], rhs,
                             start=True, stop=False)

        def bank_close(ap, ncols, dep=None, npart=128):
            rhs = z1[0:1, 0:ncols] if dep is None else dep
            lhs = zf if rhs.dtype == F32 else z1
            nc.tensor.matmul(ap, lhs[0:1, 0:npart], rhs,
                             start=False, stop=True)

        # -------- setup: enc_projT (+enc_b+dec_b), gates-x precompute ------
        with tc.tile_pool(name="se", bufs=1) as se, \
             tc.tile_pool(name="sep", bufs=2, space="PSUM") as sep:
            et = se.tile([128, 4, BN], F16)
            ewt = se.tile([128, 4, A], F16)
            nc.sync.dma_start(
                out=ewt[:, :, :],
                in_=d_ewt[:, :].rearrange("(k p) a -> p k a", p=128))
            for ch in range(4):                      # chunked: MMs start early
                nc.sync.dma_start(
                    out=et[:, :, ch * 392:(ch + 1) * 392],
                    in_=d_enc_t[:, ch * 392:(ch + 1) * 392].rearrange(
                        "(k p) n -> p k n", p=128))
            nc.sync.dma_start(
                out=dwt[:, :, :],
                in_=d_dwt[:, :].rearrange("(k p) a -> p k a", p=128))
            nc.sync.dma_start(
                out=emb[:, 0:4, :],
                in_=d_emb[0:512, :].rearrange("(k p) t -> p k t", p=128))
            nc.sync.dma_start(out=emb[0:1, 4, :], in_=d_emb[512:513, :])
            nc.sync.dma_start(
                out=wx[:, 0:4, :],
                in_=d_wx[0:512, :].rearrange("(k p) g -> p k g", p=128))
            nc.sync.dma_start(out=wx[0:1, 4, :], in_=d_wx[512:513, :])
            nc.sync.dma_start(out=enr[:, :, :],
                              in_=d_enc_r[:, :, :].rearrange("j p h -> p j h"))
            # wc: ctx half (k 0:4) first (needed at t0), then h half
            nc.sync.dma_start(
                out=wc[:, 0:4, :],
                in_=d_wc[0:512, :].rearrange("(k p) g -> p k g", p=128))
            nc.sync.dma_start(
                out=wc[:, 4:8, :],
                in_=d_wc[512:1024, :].rearrange("(k p) g -> p k g", p=128))
            for q in range(4):                    # whole fc_W, fp8 resident
                nc.sync.dma_start(
                    out=fcpre[:, :, q * 5000:(q + 1) * 5000],
                    in_=d_fcw[:, q * 5000:(q + 1) * 5000].rearrange(
                        "(k p) v -> p k v", p=128))
            for at in range(4):                      # a-tile = out m-tile
                for ch in range(4):                  # 1568 = 4*392
                    pt = sep.tile([128, 392], F32, tag="sep")
                    for k in range(4):
                        nc.tensor.matmul(
                            pt[:, :],
                            ewt[:, k, at * 128:(at + 1) * 128],
                            et[:, k, ch * 392:(ch + 1) * 392],
                            start=(k == 0), stop=(k == 3))
                    # alternate the bias-add drain across DVE and ACT so
                    # neither engine serializes the whole ept production
                    osl = ept[:, at, ch * 392:(ch + 1) * 392]
                    if ch % 2 == 0:
                        nc.vector.tensor_scalar_add(
                            out=osl, in0=pt[:, :],
                            scalar1=ebdb[:, at:at + 1])
                    else:
                        nc.scalar.activation(
                            out=osl, in_=pt[:, :],
                            func=mybir.ActivationFunctionType.Identity,
                            bias=ebdb[:, at:at + 1])
            # gates-x: gx[:, g, :] = (wihxb^T emb_cat); only the first 5
            # steps' columns here -- the rest weaves into steps 0-1
            for g in range(16):
                gsl = slice(g * 128, (g + 1) * 128)
                ptg = sep.tile([128, 40], F32, tag="sep")
                for k in range(5):
                    kr = 128 if k < 4 else 1
                    nc.tensor.matmul(ptg[:, :], wx[0:kr, k, gsl],
                                     emb[0:kr, k, 0:40],
                                     start=(k == 0), stop=(k == 4))
                nc.vector.tensor_copy(out=gx[:, g, 0:40], in_=ptg[:, :])

        # -------- recurrence, 2-way b-split software pipeline --------
        # Groups G0 = b 0..3, G1 = b 4..7 run their serial attention/LSTM
        # chains staggered so ACT's tanh burst for one group overlaps the
        # other group's softmax/ctx/gates glue. fc m0 chunks weave into PE
        # idle at t>=16 (1-bank jobs, 2 PSUM bufs ping-pong).
        m0_jobs = list(range(NCH))

        with tc.tile_pool(name="pdx0", bufs=1, space="PSUM") as pdx0, \
             tc.tile_pool(name="pdx1", bufs=1, space="PSUM") as pdx1, \
             tc.tile_pool(name="pg0", bufs=1, space="PSUM") as pg0, \
             tc.tile_pool(name="pg1", bufs=1, space="PSUM") as pg1, \
             tc.tile_pool(name="ps0", bufs=1, space="PSUM") as ps0, \
             tc.tile_pool(name="ps1", bufs=1, space="PSUM") as ps1, \
             tc.tile_pool(name="pf", bufs=2, space="PSUM") as pfp, \
             tc.tile_pool(name="stp", bufs=2) as stp, \
             tc.tile_pool(name="fo", bufs=3) as fop, \
             tc.tile_pool(name="xp", bufs=1) as xp:

            pdx = [pdx0, pdx1]
            pgt = [pg0, pg1]
            psp = [ps0, ps1]
            fc_pending = []

            def m0_mm(ch):
                pf = pfp.tile([128, VC], F32, tag="pf", name=f"pf{ch}")
                for k in range(4):
                    nc.tensor.matmul(
                        pf[:, :], hallt[:, k, 0:128],
                        fcpre[:, k, ch * VC:(ch + 1) * VC],
                        start=(k == 0), stop=(k == 3))
                fc_pending.append((pf, ch))

            def m0_drain(eng, limit=None):
                nd = len(fc_pending) if limit is None else \
                    min(limit, len(fc_pending))
                for _ in range(nd):
                    pf, ch = fc_pending.pop(0)
                    fo = fop.tile([128, VC], F16, tag="fo", name=f"fo{ch}")
                    if eng is nc.scalar:
                        eng.copy(out=fo[:, :], in_=pf[:, :])
                    else:
                        eng.tensor_copy(out=fo[:, :], in_=pf[:, :])
                    nc.sync.dma_start(
                        out=d_out[:, ch * VC:(ch + 1) * VC], in_=fo[:, :])

            def gx_pass(g0, g1, c0, c1):
                for g in range(g0, g1):
                    gsl = slice(g * 128, (g + 1) * 128)
                    ptg = pfp.tile([128, c1 - c0], F32, tag="pf",
                                   name=f"gxb{g}_{c0}")
                    for k in range(5):
                        kr = 128 if k < 4 else 1
                        nc.tensor.matmul(ptg[:, :], wx[0:kr, k, gsl],
                                         emb[0:kr, k, c0:c1],
                                         start=(k == 0), stop=(k == 4))
                    nc.vector.tensor_copy(out=gx[:, g, c0:c1], in_=ptg[:, :])

            GW = 4 * N  # 784: one b-group's column width in (b n) layouts

            def g_dec(i, t):
                """dec matmuls for group i -> dps_i [128, 4at, 4b]."""
                dps = pdx[i].tile([128, 4, 4], F32, tag="dx",
                                  name=f"dec{t}_{i}")
                hsl = slice(t * 8 + 4 * i - 8, t * 8 + 4 * i - 4)
                bank_open(flat(dps, 16), 16,
                          dep=hallt[0:1, :, hsl])
                for at in range(4):
                    for k in range(4):
                        nc.tensor.matmul(
                            dps[:, at, :],
                            dwt[:, k, at * 128:(at + 1) * 128],
                            hallt[:, k, hsl], start=False, stop=False)
                bank_close(flat(dps, 16), 16)
                return dps

            def g_gates_h(i, t):
                gps = pgt[i].tile([128, 16, 4], F32, tag="g",
                                  name=f"g{t}_{i}")
                hsl = slice(t * 8 + 4 * i - 8, t * 8 + 4 * i - 4)
                xsl = slice(t * 8 + 4 * i, t * 8 + 4 * i + 4)
                bank_open(flat(gps, 64), 64)
                for g in range(16):
                    gsl = slice(g * 128, (g + 1) * 128)
                    if t > 0:
                        for k in range(4):
                            nc.tensor.matmul(gps[:, g, :], wc[:, 4 + k, gsl],
                                             hallt[:, k, hsl],
                                             start=False, stop=False)
                    nc.tensor.matmul(gps[:, g, :], id128[:, :],
                                     gx[:, g, xsl], start=False, stop=False)
                return gps

            def g_adds(i, t, dps):
                """ept + dec broadcast: DVE at0/at2 (PSUM direct), Pool
                at1/at3 (via dect copies)."""
                dect = stp.tile([128, 2, 4], F32, tag=f"dect{i}",
                                name=f"dect{t}_{i}")
                nc.vector.tensor_copy(out=dect[:, 0, :], in_=dps[:, 1, :])
                nc.vector.tensor_copy(out=dect[:, 1, :], in_=dps[:, 3, :])
                xas = []
                for ap in range(2):
                    xa = xp.tile([128, 2, GW], F16, tag=f"x{i}_{ap}",
                                 name=f"xa{t}_{i}{ap}")
                    xas.append(xa)
                for at in (1, 0, 3, 2):
                    ap, j = at // 2, at % 2
                    src = ept[:, at, i * GW:(i + 1) * GW].rearrange(
                        "p (b n) -> p b n", n=N)
                    dst = xas[ap][:, j, :].rearrange("p (b n) -> p b n", n=N)
                    if j == 0:
                        nc.vector.tensor_add(
                            out=dst, in0=src, in1=_bcast(dps[:, at, :], N))
                    else:
                        nc.gpsimd.tensor_add(
                            out=dst, in0=src,
                            in1=_bcast(dect[:, at // 2, :], N))
                return xas

            def g_tanh_scores(i, t, xas):
                psc = psp[i].tile([4, N], F32, tag="sc", name=f"sc{t}_{i}")
                for ap in range(2):
                    if t == 0:
                        nc.scalar.activation(
                            out=xt8[:, 2 * ap:2 * ap + 2,
                                    i * GW:(i + 1) * GW],
                            in_=ept[:, 2 * ap:2 * ap + 2,
                                    i * GW:(i + 1) * GW], func=TANH)
                    else:
                        nc.scalar.activation(
                            out=xt8[:, 2 * ap:2 * ap + 2,
                                    i * GW:(i + 1) * GW],
                            in_=xas[ap][:, :, :], func=TANH)
                for ap in range(2):
                    for bl in range(4):
                        nc.tensor.matmul(
                            psc[:, :],
                            ewm8[:, ap, :, i, bl * 4:(bl + 1) * 4],
                            xt8[:, 2 * ap:2 * ap + 2,
                                i * GW + bl * N:i * GW + (bl + 1) * N],
                            start=(ap == 0 and bl == 0),
                            stop=(ap == 1 and bl == 3),
                            perf_mode=DR)
                return psc

            def g_softmax(i, t, psc):
                atw = stp.tile([4, N], F16, tag=f"atw{i}")
                zs = stp.tile([4, 1], F32, tag=f"zs{i}")
                nc.scalar.activation(out=atw[:, :], in_=psc[:, :],
                                     func=EXP, scale=1.0 / EWS,
                                     accum_out=zs[:, 0:1])
                rz = stp.tile([4, 1], F32, tag=f"rz{i}")
                nc.vector.reciprocal(out=rz[:, :], in_=zs[:, :])
                atwn = stp.tile([4, N], F16, tag=f"atwn{i}")
                nc.vector.tensor_scalar_mul(out=atwn[:, :], in0=atw[:, :],
                                            scalar1=rz[:, :])
                return atwn

            def g_ctx(i, t, atwn):
                p12 = psp[i].tile([128, 2, 4], F16, tag="sc",
                                  name=f"tp{t}_{i}")
                nc.tensor.transpose(p12[:, 0, :], atwn[:, 0:128],
                                    id8[0:4, 0:4])
                nc.tensor.transpose(p12[0:N - 128, 1, :], atwn[:, 128:N],
                                    id8[0:4, 0:4])
                awt = stp.tile([128, 2, 4], F16, tag=f"awt{i}")
                nc.vector.tensor_copy(out=awt[:, 0, :], in_=p12[:, 0, :])
                nc.vector.tensor_copy(out=awt[0:N - 128, 1, :],
                                      in_=p12[0:N - 128, 1, :])
                cps = pdx[i].tile([128, 4, 4], F32, tag="dx",
                                  name=f"ctx{t}_{i}")
                bank_open(flat(cps, 16), 16, dep=atwn[0:1, 0:16])
                for b in range(4):
                    bb = 4 * i + b
                    for at in range(4):
                        asl = slice(at * 128, (at + 1) * 128)
                        nc.tensor.matmul(
                            cps[:, at, b:b + 1], enr[0:128, 2 * bb, asl],
                            awt[0:128, 0, b:b + 1],
                            start=False, stop=False)
                        nc.tensor.matmul(
                            cps[:, at, b:b + 1],
                            enr[0:N - 128, 2 * bb + 1, asl],
                            awt[0:N - 128, 1, b:b + 1],
                            start=False, stop=False)
                bank_close(flat(cps, 16), 16)
                ctxt = stp.tile([128, 4, 4], F16, tag=f"ctxt{i}")
                nc.vector.tensor_copy(out=ctxt[:, :, :], in_=cps[:, :, :])
                return ctxt

            def g_gates_ctx(i, gps, ctxt):
                for g in range(16):
                    gsl = slice(g * 128, (g + 1) * 128)
                    for k in range(4):
                        nc.tensor.matmul(gps[:, g, :], wc[:, k, gsl],
                                         ctxt[:, k, :],
                                         start=False, stop=False)
                bank_close(flat(gps, 64), 64)

            def g_pointwise(i, t, gps):
                csl = slice(4 * i, 4 * i + 4)
                th = stp.tile([128, 16, 4], F16, tag=f"th{i}")
                nc.scalar.activation(out=th[:, :, :], in_=gps[:, :, :],
                                     func=TANH, scale=0.5)
                a2 = stp.tile([128, 4, 4], F32, tag=f"a2{i}")
                nc.vector.scalar_tensor_tensor(
                    out=a2[:, :, :], in0=th[:, 4:8, :], scalar=1.0,
                    in1=c2[:, :, csl], op0=ADD, op1=MULT)
                bb = stp.tile([128, 4, 4], F32, tag=f"bb{i}")
                nc.vector.scalar_tensor_tensor(
                    out=bb[:, :, :], in0=th[:, 0:4, :], scalar=1.0,
                    in1=th[:, 12:16, :], op0=ADD, op1=MULT)
                nc.vector.scalar_tensor_tensor(
                    out=c2[:, :, csl], in0=a2[:, :, :], scalar=0.5,
                    in1=bb[:, :, :], op0=MULT, op1=ADD)
                thc = stp.tile([128, 4, 4], F16, tag=f"thc{i}")
                nc.scalar.activation(out=thc[:, :, :], in_=c2[:, :, csl],
                                     func=TANH, scale=0.5)
                nc.vector.scalar_tensor_tensor(
                    out=hallt[:, :, t * 8 + 4 * i:t * 8 + 4 * i + 4],
                    in0=th[:, 8:12, :],
                    scalar=1.0, in1=thc[:, :, :], op0=ADD, op1=MULT)

            for t in range(T):
                # dec + adds for both groups lead; gates-h fills PE while
                # the adds run on DVE/Pool
                dps = [None, None]
                gps = [None, None]
                xas = [None, None]
                for i in range(2):
                    if t > 0:
                        dps[i] = g_dec(i, t)
                    gps[i] = g_gates_h(i, t)
                    if t > 0:
                        xas[i] = g_adds(i, t, dps[i])
                # weave slot 1
                if t >= 16 and m0_jobs:
                    m0_mm(m0_jobs.pop(0))
                    m0_mm(m0_jobs.pop(0))
                if t < 4:
                    gx_pass(4 * t, 4 * t + 4, 40, 152)
                psc0 = g_tanh_scores(0, t, xas[0])
                if t >= 16:
                    m0_drain(nc.vector, limit=1)
                atwn0 = g_softmax(0, t, psc0)
                psc1 = g_tanh_scores(1, t, xas[1])
                ctxt0 = g_ctx(0, t, atwn0)
                g_gates_ctx(0, gps[0], ctxt0)
                if t >= 16 and m0_jobs:
                    m0_mm(m0_jobs.pop(0))
                    m0_mm(m0_jobs.pop(0))
                    m0_drain(nc.vector, limit=1)
                atwn1 = g_softmax(1, t, psc1)
                # weave slot 2
                if t >= 16 and m0_jobs:
                    m0_mm(m0_jobs.pop(0))
                    m0_mm(m0_jobs.pop(0))
                ctxt1 = g_ctx(1, t, atwn1)
                g_pointwise(0, t, gps[0])
                g_gates_ctx(1, gps[1], ctxt1)
                if t >= 16:
                    m0_drain(nc.scalar, limit=1)
                    m0_drain(nc.vector, limit=1)
                g_pointwise(1, t, gps[1])
                if t >= 16 and m0_jobs:
                    m0_mm(m0_jobs.pop(0))
                    m0_mm(m0_jobs.pop(0))
                    m0_drain(nc.scalar, limit=1)
                    m0_drain(nc.vector, limit=1)

            m0_drain(nc.vector)

        # -------- tail: m1 transposed per group + m0 leftovers --------
        with tc.tile_pool(name="fom", bufs=3) as fom, \
             tc.tile_pool(name="pf2", bufs=2, space="PSUM") as pf2, \
             tc.tile_pool(name="pt1", bufs=2, space="PSUM") as pt1p:

            def m1t_group(g, eng):
                pt1 = pt1p.tile([VT, 16, 24], F32, tag="pt1", name=f"pt1{g}")
                bank_open(pt1[:, 0, 0:1], 1, npart=VT)
                for v in range(16):
                    for k in range(4):
                        nc.tensor.matmul(
                            pt1[:, v, :],
                            fcpre[:, k, g * 16 * VT + v * VT:
                                  g * 16 * VT + (v + 1) * VT],
                            hallt[:, k, 128:152],
                            start=False, stop=False)
                bank_close(pt1[:, 0, 0:1], 1, npart=VT)
                fo1 = fom.tile([VT, 16, 24], F16, tag="fo1", name=f"fo1{g}")
                if eng is nc.scalar:
                    eng.copy(out=fo1[:, :, :], in_=pt1[:, :, :])
                    dma = nc.sync
                else:
                    eng.tensor_copy(out=fo1[:, :, :], in_=pt1[:, :, :])
                    dma = nc.scalar
                dma.dma_start(
                    out=d_outT[g * 16 * VT:(g + 1) * 16 * VT, :].rearrange(
                        "(v p) c -> p v c", p=VT),
                    in_=fo1[:, :, :])

            def m0_chunk2(c0, eng):
                # leftover chunks in aligned consecutive pairs; 512-wide
                # slots keep each chunk's matmul output bank-aligned
                pf = pf2.tile([128, 2, 512], F32, tag="pf2", name=f"pfb{c0}")
                for cc in range(2):
                    for k in range(4):
                        nc.tensor.matmul(
                            pf[:, cc, 0:VC], hallt[:, k, 0:128],
                            fcpre[:, k, (c0 + cc) * VC:(c0 + cc + 1) * VC],
                            start=(k == 0), stop=(k == 3))
                fo = fom.tile([128, 2, VC], F16, tag="fo2", name=f"fob{c0}")
                if eng is nc.scalar:
                    eng.copy(out=fo[:, :, :], in_=pf[:, :, 0:VC])
                    dma = nc.sync
                else:
                    eng.tensor_copy(out=fo[:, :, :], in_=pf[:, :, 0:VC])
                    dma = nc.scalar
                dma.dma_start(
                    out=d_out[:, c0 * VC:(c0 + 2) * VC],
                    in_=fo[:, :, :].rearrange("p a b -> p (a b)"))

            assert len(m0_jobs) % 2 == 0 and m0_jobs == sorted(m0_jobs)
            pairs = [m0_jobs[j] for j in range(0, len(m0_jobs), 2)]
            engs = [nc.vector, nc.scalar]
            i = 0
            for g in range(NFG):
                m1t_group(g, engs[i % 2]); i += 1
                while pairs and len(pairs) > (NFG - 1 - g) * 14 // 10:
                    m0_chunk2(pairs.pop(0), engs[i % 2]); i += 1


_PROGRAM = None


def kernel(**inputs) -> np.ndarray:
    global _PROGRAM
    if _PROGRAM is None:
        _PROGRAM = build_program()
    in_maps = [prep_core(c, inputs) for c in range(NC)]
    res = run_bass_kernel_spmd(_PROGRAM, in_maps, core_ids=list(range(NC)))
    fcb = np.asarray(inputs["fc_b"], np.float32)
    out = np.zeros((B, L, V), np.float32)
    for c in range(NC):
        lg0 = res.results[c]["logits"].astype(np.float32) / FCS  # [128, V]
        lg0 = lg0.reshape(16, BS, V)
        out[c * BS:(c + 1) * BS, 1:17, :] = lg0.transpose(1, 0, 2) + fcb
        lgT = res.results[c]["logitsT"].astype(np.float32) / FCS  # [V, 24]
        lg1 = lgT.T.reshape(3, BS, V)
        out[c * BS:(c + 1) * BS, 17:, :] = lg1.transpose(1, 0, 2) + fcb
    return out


if __name__ == "__main__":
    import reference
    ins = {k: np.asarray(v) for k, v in reference.setup_inputs().items()}
    got = kernel(**ins)
    exp = np.asarray(reference.reference(**reference.setup_inputs()))
    err = np.abs(got - exp).max() / (np.abs(exp).max() + 1e-12)
    print("Relative error:", err)
